# revision 7
# baseline (speedup 1.0000x reference)
"""Trainium2 Bass kernel for nn_Contrast2 (contrastive pixel loss).

Strategy (pure data parallelism per the sharding hint):
  - B=24 batches are sharded 3-per-core across 8 NeuronCores.
  - The reference only ever reads the three [B,C,H,W] projection tensors at
    S=5 sampled spatial positions per batch (via `indices`).  The host side
    of this kernel performs that index-selection while building each core's
    shard: core k receives exactly the 3*S C-vectors it needs from each
    projection, packed with the (constant) block-diag mask and identity into
    a single [15, 222] f32 tile.
  - The device program (identical SPMD program on all 8 cores) does all the
    floating-point math of the loss: L2 norms + clipped normalization,
    positive-pair dot products, the SxS cosine-similarity Gram matrix via
    the tensor engine, exp(g/tau), masked negative sums, and the final
    log-ratio per sample.  Each core returns its 15 per-sample losses.
  - Host combines: mean over S per batch, sum over batches / B  (the
    "all-reduce mean" of the hint, done on 120 scalars).
"""

import numpy as np

import concourse.bass as bass
import concourse.tile as tile
from concourse import bacc, mybir
from concourse.bass_utils import run_bass_kernel_spmd

TAU = 0.07
EPS = 1e-8
NORM_EPS = 1e-12
N_CORES = 8
C = 64  # channel dim

# Set by tests to request an NTFF profile of the device program; the last
# BassKernelResults lands in LAST_RESULTS.
PROFILE = False
LAST_RESULTS = None

_PROGRAM_CACHE = {}


def _build_program(rows, width):
    """Per-core device program.  rows = Bc*S sample-vectors on partitions;
    xin columns = [c | p1 | p2 | mask(rows) | identity(rows)]."""
    f32 = mybir.dt.float32
    mult = mybir.AluOpType.mult
    add = mybir.AluOpType.add
    sub = mybir.AluOpType.subtract
    Act = mybir.ActivationFunctionType

    nc = bacc.Bacc("TRN2", target_bir_lowering=False, debug=False,
                   num_devices=N_CORES)
    xin_d = nc.dram_tensor("xin", [rows, width], f32, kind="ExternalInput").ap()
    out_d = nc.dram_tensor("out", [rows, 1], f32, kind="ExternalOutput").ap()

    with tile.TileContext(nc) as tc:
        with tc.tile_pool(name="sb", bufs=1) as sb, \
             tc.tile_pool(name="ps", bufs=1, space="PSUM") as ps:
            X = sb.tile([rows, width], f32)
            nc.sync.dma_start(X[:], xin_d[:])
            x = X[:, 0:3 * C]                      # [R, 192]
            mask = X[:, 3 * C:3 * C + rows]        # [R, R]
            ident = X[:, 3 * C + rows:3 * C + 2 * rows]  # [R, R]

            # squared norms of c / p1 / p2 per row  -> [R, 3]
            sq = sb.tile([rows, 3 * C], f32)
            nc.scalar.square(sq[:], x)
            sumsq = sb.tile([rows, 3], f32)
            nc.vector.reduce_sum(sumsq[:], sq.rearrange("p (g c) -> p g c", g=3),
                                 axis=mybir.AxisListType.X)
            nrm = sb.tile([rows, 3], f32)
            nc.scalar.sqrt(nrm[:], sumsq[:])
            nrmc = sb.tile([rows, 3], f32)
            nc.vector.tensor_scalar_max(nrmc[:], nrm[:], NORM_EPS)
            inv = sb.tile([rows, 3], f32)
            nc.vector.reciprocal(inv[:], nrmc[:])

            # positive-pair raw dots, then scale by 1/(|c||pi|)
            prod1 = sb.tile([rows, C], f32)
            nc.vector.tensor_tensor(prod1[:], x[:, 0:C], x[:, C:2 * C], mult)
            d1r = sb.tile([rows, 1], f32)
            nc.vector.reduce_sum(d1r[:], prod1[:], axis=mybir.AxisListType.X)
            prod2 = sb.tile([rows, C], f32)
            nc.vector.tensor_tensor(prod2[:], x[:, 0:C], x[:, 2 * C:3 * C], mult)
            d2r = sb.tile([rows, 1], f32)
            nc.vector.reduce_sum(d2r[:], prod2[:], axis=mybir.AxisListType.X)
            d1 = sb.tile([rows, 1], f32)
            nc.vector.tensor_scalar(d1[:], d1r[:], inv[:, 0:1], inv[:, 1:2],
                                    op0=mult, op1=mult)
            d2 = sb.tile([rows, 1], f32)
            nc.vector.tensor_scalar(d2[:], d2r[:], inv[:, 0:1], inv[:, 2:3],
                                    op0=mult, op1=mult)
            sumd = sb.tile([rows, 1], f32)
            nc.vector.tensor_tensor(sumd[:], d1[:], d2[:], add)

            # normalized current view and its Gram matrix
            chat = sb.tile([rows, C], f32)
            nc.vector.tensor_scalar_mul(chat[:], x[:, 0:C], inv[:, 0:1])
            chatT_ps = ps.tile([C, rows], f32)
            nc.tensor.transpose(chatT_ps[:], chat[:], ident)
            chatT = sb.tile([C, rows], f32)
            nc.scalar.copy(chatT[:], chatT_ps[:])
            gram = ps.tile([rows, rows], f32)
            nc.tensor.matmul(gram[:], chatT[:], chatT[:], start=True, stop=True)

            # E = exp(g/tau); negatives = sum over same-batch, t != s
            E = sb.tile([rows, rows], f32)
            nc.scalar.activation(E[:], gram[:], Act.Exp, scale=1.0 / TAU)
            Em = sb.tile([rows, rows], f32)
            nc.vector.tensor_tensor(Em[:], E[:], mask, mult)
            neg = sb.tile([rows, 1], f32)
            nc.vector.reduce_sum(neg[:], Em[:], axis=mybir.AxisListType.X)

            # loss = log(pos + neg + eps) - (d1+d2)/tau,  pos = exp((d1+d2)/tau)
            pos = sb.tile([rows, 1], f32)
            nc.scalar.activation(pos[:], sumd[:], Act.Exp, scale=1.0 / TAU)
            den = sb.tile([rows, 1], f32)
            nc.vector.tensor_tensor(den[:], neg[:], pos[:], add)
            den_eps = sb.tile([rows, 1], f32)
            nc.vector.tensor_scalar_add(den_eps[:], den[:], EPS)
            lnden = sb.tile([rows, 1], f32)
            nc.scalar.activation(lnden[:], den_eps[:], Act.Ln)
            sdt = sb.tile([rows, 1], f32)
            nc.scalar.mul(sdt[:], sumd[:], 1.0 / TAU)
            loss = sb.tile([rows, 1], f32)
            nc.vector.tensor_tensor(loss[:], lnden[:], sdt[:], sub)

            nc.sync.dma_start(out_d[:], loss[:])
    nc.compile()
    return nc


def _get_program(rows, width):
    key = (rows, width)
    if key not in _PROGRAM_CACHE:
        _PROGRAM_CACHE[key] = _build_program(rows, width)
    return _PROGRAM_CACHE[key]


def _pack_inputs(proj0, proj1, proj2, idx, indices):
    """Host-side shard prep: gather the sampled C-vectors and pack per-core
    tiles.  Returns (in_maps, B, S)."""
    B, Cc, H, W = proj0.shape
    assert Cc == C
    S = indices.shape[1]
    projs = [proj0, proj1, proj2]
    i = int(idx)
    order = [projs[i]] + [p for j, p in enumerate(projs) if j != i]

    idx3 = np.ascontiguousarray(indices.astype(np.int64))[:, None, :]  # [B,1,S]
    gath = []
    for p in order:
        flat = p.reshape(B, Cc, H * W)
        g = np.take_along_axis(flat, idx3, axis=2)      # [B,C,S]
        gath.append(np.ascontiguousarray(g.transpose(0, 2, 1)))  # [B,S,C]

    assert B % N_CORES == 0
    Bc = B // N_CORES
    rows = Bc * S
    width = 3 * C + 2 * rows

    blockmask = (np.kron(np.eye(Bc, dtype=np.float32), np.ones((S, S), np.float32))
                 - np.eye(rows, dtype=np.float32))
    ident = np.eye(rows, dtype=np.float32)

    in_maps = []
    for k in range(N_CORES):
        xin = np.empty((rows, width), np.float32)
        sl = slice(k * Bc, (k + 1) * Bc)
        for j in range(3):
            xin[:, j * C:(j + 1) * C] = gath[j][sl].reshape(rows, Cc)
        xin[:, 3 * C:3 * C + rows] = blockmask
        xin[:, 3 * C + rows:] = ident
        in_maps.append({"xin": xin})
    return in_maps, B, S, rows, width


def kernel(proj0, proj1, proj2, idx, pseudo_label, mask, indices, sample_num):
    global LAST_RESULTS
    in_maps, B, S, rows, width = _pack_inputs(proj0, proj1, proj2, idx, indices)
    nc = _get_program(rows, width)
    res = run_bass_kernel_spmd(nc, in_maps, list(range(N_CORES)),
                               trace=bool(PROFILE))
    LAST_RESULTS = res
    per_sample = np.stack([res.results[k]["out"].reshape(rows)
                           for k in range(N_CORES)])          # [8, Bc*S]
    per_sample = per_sample.reshape(B, S).astype(np.float64)
    total = per_sample.mean(axis=1).sum() / B
    return np.float32(total)
